# revision 63
# baseline (speedup 1.0000x reference)
"""Transformer block (LN -> 12-head causal attention -> residual -> LN -> MLP
-> residual) for B=4, T=2048, C=768 on 8 trn2 NeuronCores.

Sharding: core = (batch, token-half). Each core handles one batch's K/V in
full and produces the final output for half the tokens (even or odd 128-token
blocks, balancing the causal triangle). No collectives; per-core structural
differences are carried in input *data* (host-gathered xTm, causal-boundary
mask tiles, even/odd select vector) so a single SPMD program runs on all 8
cores.

Engine plan (from perfetto/NTFF trace analysis; 751us baseline -> ~490us):
- QKV: one streaming pass over x. Scalar engine casts fp32->bf16, DVE
  squares in bf16 (2x mode); tensor engine does LN-stats matmuls + K/Q/V
  GEMMs; DVE applies the LN affine in place (xh becomes h). LN finalize uses
  reciprocal_approx_fast + scalar Sqrt. Q's token subset comes from an
  even/odd data-driven blend of the full-T LN coefficients applied to
  host-gathered xTm (keeps the program SPMD across cores).
- Attention: sb-outer loop; per key block one paired exp covers both query
  groups (two QK matmuls into a 2-bank PSUM tile). QK->exp->PV is
  software-pipelined (PV lags 3 jobs) so the in-order tensor queue never
  stalls on the scalar exp and the PE stays in its ramped p-state; causal
  boundary masks are multiplicative 0/1 tiles applied to wei on the
  otherwise-idle gpsimd engine; the row-sum rides the PV matmul via a ones
  column in V; normalization is one batched approx reciprocal + paired
  tensor-engine broadcasts at the end.
- MLP: w1 DMA prefetched at attention start, w2 at proj start (pool-release
  deps give the timing, right-side SBUF pools make the lifetimes legal);
  LN2 stats interleave into proj with a one-tile lag; relu+bias on scalar.
"""

import math
import os
import sys

for _p in ("/opt/trn_rl_repo", "/root/.axon_site/_ro/trn_rl_repo"):
    if os.path.isdir(_p) and _p not in sys.path:
        sys.path.append(_p)

import numpy as np
import ml_dtypes

import concourse.bacc as bacc
import concourse.tile as tile
import concourse.mybir as mybir
from concourse import bass_utils
from concourse.alu_op_type import AluOpType

BF = mybir.dt.bfloat16
FP = mybir.dt.float32
AF = mybir.ActivationFunctionType

B, T, C, H, HD = 4, 2048, 768, 12, 64
EPS = 1e-5
SHIFT = 40.0  # constant softmax shift: exp(s - SHIFT); exact softmax
NP = C // 128  # 6 feature partition-tiles
NT = T // 128  # 16 token blocks
TM = T // 2    # 1024 tokens owned per core
bf16 = ml_dtypes.bfloat16

_cache = {}


def _build():
    nc = bacc.Bacc("TRN2", target_bir_lowering=False, debug=False)
    d_xT = nc.dram_tensor("xT", [C, T], FP, kind="ExternalInput").ap()
    d_xTm = nc.dram_tensor("xTm", [C, TM], FP, kind="ExternalInput").ap()
    d_wqkvp = nc.dram_tensor("wqkvp", [C, 4 * C], BF, kind="ExternalInput").ap()
    d_w1 = nc.dram_tensor("w1p", [C, 4 * C], BF, kind="ExternalInput").ap()
    d_w2 = nc.dram_tensor("w2p", [C, 4 * C], BF, kind="ExternalInput").ap()
    d_bias = nc.dram_tensor("biasp", [C, 9], FP, kind="ExternalInput").ap()
    d_bvrow = nc.dram_tensor("bvrow", [1, C], FP, kind="ExternalInput").ap()
    d_masks = nc.dram_tensor("masks", [384, 128], BF, kind="ExternalInput").ap()
    d_sel = nc.dram_tensor("sel", [128, 2], FP, kind="ExternalInput").ap()
    d_out = nc.dram_tensor("outT", [C, TM], FP, kind="ExternalOutput").ap()

    with tile.TileContext(nc) as tc:
        _body(nc, tc, d_xT, d_xTm, d_wqkvp, d_w1, d_w2, d_bias, d_bvrow,
              d_masks, d_sel, d_out)
    nc.compile()
    return nc


def _body(nc, tc, d_xT, d_xTm, d_wqkvp, d_w1, d_w2, d_bias, d_bvrow,
          d_masks, d_sel, d_out):
    from contextlib import ExitStack

    es = ExitStack()
    g_const = es.enter_context(tc.tile_pool(name="const", bufs=1))
    w_es = ExitStack()
    g_w = w_es.enter_context(tc.tile_pool(name="wqkvp", bufs=1))
    attnT_es = ExitStack()
    g_attnT = attnT_es.enter_context(tc.tile_pool(name="attnT", bufs=1))
    attnT = [g_attnT.tile([128, TM], BF, tag=f"aT{i}", name=f"aT{i}") for i in range(NP)]
    kqv_es = ExitStack()
    g_kqv = kqv_es.enter_context(tc.tile_pool(name="kqv", bufs=1))

    # ---- weights for attention part (vector DMA queue: off the x path,
    # and off gpsimd whose queue must stay free for LN broadcasts) ----
    w_sb = [g_w.tile([128, 4 * C], BF, tag=f"w{c}", name=f"w{c}") for c in range(NP)]
    for c in range(NP):
        nc.gpsimd.dma_start(w_sb[c][:], d_wqkvp[c * 128:(c + 1) * 128, :])

    # ---- constants ----
    ones_bf = g_const.tile([128, 1], BF, tag="ones_bf", name="ones_bf")
    nc.vector.memset(ones_bf[:], 1.0)
    ones64_bc = g_const.tile([128, 64], BF, tag="ones64_bc", name="ones64_bc")
    nc.vector.memset(ones64_bc[:], 1.0)
    eps_c = g_const.tile([128, 1], FP, tag="eps_c", name="eps_c")
    nc.vector.memset(eps_c[:], EPS)
    shift_c = g_const.tile([128, 1], FP, tag="shift_c", name="shift_c")
    nc.vector.memset(shift_c[:], -SHIFT)
    bias_sb = [g_const.tile([128, 9], FP, tag=f"bias{f}", name=f"bias{f}") for f in range(NP)]
    # multiplicative causal-mask tiles (1 allowed / 0 masked) + identity;
    # DMAs for these consts are emitted inside the QKV flow (sync queue) at
    # the point each is first needed, keeping early queues clear
    mask_a = g_const.tile([128, 128], BF, tag="mask_a", name="mask_a")
    mask_b = g_const.tile([128, 128], BF, tag="mask_b", name="mask_b")
    ident = g_const.tile([128, 128], BF, tag="ident", name="ident")
    sel_sb = g_const.tile([128, 2], FP, tag="sel", name="sel")
    bv_row = g_const.tile([1, C], FP, tag="bv_row", name="bv_row")
    bv_rb = g_const.tile([1, C], BF, tag="bv_rb", name="bv_rb")
    bv_bc = g_const.tile([128, C], BF, tag="bv_bc", name="bv_bc")

    # ---- persistent activation storage ----
    KT = [g_kqv.tile([128, T], BF, tag=f"KT{i}", name=f"KT{i}") for i in range(NP)]
    QT = [g_kqv.tile([128, TM], BF, tag=f"QT{i}", name=f"QT{i}") for i in range(NP)]
    Vsb = [g_kqv.tile([128, H * 65], BF, tag=f"V{t}", name=f"V{t}") for t in range(NT)]

    # ================= LN1 + QKV (single streaming pass) =================
    ln_es = ExitStack()
    g_xh = ln_es.enter_context(tc.tile_pool(name="xh", bufs=1, named_scope="qkv"))
    g_roll = ln_es.enter_context(tc.tile_pool(name="lnroll", bufs=2))
    g_bc = ln_es.enter_context(tc.tile_pool(name="lnbc", bufs=1))
    g_small = ln_es.enter_context(tc.tile_pool(name="lnsmall", bufs=1))
    sps_es = ExitStack()
    sps = sps_es.enter_context(tc.tile_pool(name="statps", bufs=1, space="PSUM"))
    gps_es = ExitStack()
    gps = gps_es.enter_context(tc.tile_pool(name="gemmps", bufs=4, space="PSUM"))

    # xh starts as bf16 copy of x; LN affine is applied in place -> becomes h
    xh = [g_xh.tile([128, T], BF, tag=f"xh{c}", name=f"xh{c}") for c in range(NP)]
    xhm = [g_xh.tile([128, TM], BF, tag=f"xhm{c}", name=f"xhm{c}") for c in range(NP)]
    a_bc = g_bc.tile([128, T], BF, tag="a_bc", name="a_bc")
    c_bc = g_bc.tile([128, T], BF, tag="c_bc", name="c_bc")
    am_bc = g_bc.tile([128, TM], BF, tag="am_bc", name="am_bc")
    cm_bc = g_bc.tile([128, TM], BF, tag="cm_bc", name="cm_bc")
    stats = [sps.tile([33, 512], FP, tag=f"st{g}", name=f"st{g}")
             for g in range(4)]

    def stats_panel(g):
        gsl = slice(g * 512, (g + 1) * 512)
        for c in range(NP):
            xt = g_roll.tile([128, 512], FP, tag="xt", name="xt")
            nc.sync.dma_start(xt[:], d_xT[c * 128:(c + 1) * 128, gsl])
            nc.scalar.copy(xh[c][:, gsl], xt[:])
            sq = g_roll.tile([128, 512], BF, tag="sq", name="sq")
            nc.vector.tensor_mul(sq[:], xh[c][:, gsl], xh[c][:, gsl])
            nc.tensor.matmul(stats[g][0:1, :], ones_bf[:], xh[c][:, gsl],
                             start=(c == 0), stop=(c == NP - 1),
                             skip_group_check=True)
            nc.tensor.matmul(stats[g][32:33, :], ones_bf[:], sq[:],
                             start=(c == 0), stop=(c == NP - 1),
                             skip_group_check=True)

    def finalize(g):
        gsl = slice(g * 512, (g + 1) * 512)
        mu = g_small.tile([1, 512], FP, tag="mu", name="mu")
        nc.scalar.mul(mu[:], stats[g][0:1, :], 1.0 / C)
        m2 = g_small.tile([1, 512], FP, tag="m2", name="m2")
        nc.scalar.mul(m2[:], stats[g][32:33, :], 1.0 / C)
        va = g_small.tile([1, 512], FP, tag="va", name="va")
        nc.vector.tensor_mul(va[:], mu[:], mu[:])
        nc.vector.scalar_tensor_tensor(va[:], m2[:], EPS, va[:],
                                       AluOpType.add, AluOpType.subtract)
        nc.vector.reciprocal_approx_fast(va[:], va[:])
        rstd = g_small.tile([1, 512], FP, tag="rstd", name="rstd")
        nc.scalar.activation(rstd[:], va[:], AF.Sqrt)
        nc.vector.scalar_tensor_tensor(mu[:], mu[:], -1.0, rstd[:],
                                       AluOpType.mult, AluOpType.mult)
        a5b = g_small.tile([1, 512], BF, tag="a5b", name="a5b")
        nc.vector.tensor_copy(a5b[:], rstd[:])
        c5b = g_small.tile([1, 512], BF, tag="c5b", name="c5b")
        nc.vector.tensor_copy(c5b[:], mu[:])
        nc.gpsimd.partition_broadcast(a_bc[:, gsl], a5b[:])
        nc.gpsimd.partition_broadcast(c_bc[:, gsl], c5b[:])

    def h_build(g):
        gsl = slice(g * 512, (g + 1) * 512)
        for c in range(NP):
            nc.vector.tensor_mul(xh[c][:, gsl], xh[c][:, gsl], a_bc[:, gsl])
            nc.vector.tensor_add(xh[c][:, gsl], xh[c][:, gsl], c_bc[:, gsl])

    def emit_K(g):
        gsl = slice(g * 512, (g + 1) * 512)
        for f in range(NP):
            ps = gps.tile([128, 512], FP, tag="ps", name="psk")
            for c in range(NP):
                nc.tensor.matmul(ps[:], w_sb[c][:, C + f * 128:C + (f + 1) * 128],
                                 xh[c][:, gsl], start=(c == 0), stop=(c == NP - 1))
            nc.scalar.activation(KT[f][:, gsl], ps[:], AF.Identity,
                                 bias=bias_sb[f][:, 1:2])

    def emit_V(g):
        for t in range(4 * g, 4 * g + 4):
            v3 = Vsb[t][:].rearrange("p (h d) -> p h d", d=65)
            nc.vector.memset(v3[:, :, 64:65], 1.0)
            for fs in range(2):
                n = 512 if fs == 0 else 256
                nh = n // 64
                ps = gps.tile([128, 512], FP, tag="ps", name="psv")
                for c in range(NP):
                    nc.tensor.matmul(ps[:, 0:n], xh[c][:, t * 128:(t + 1) * 128],
                                     w_sb[c][:, 2 * C + fs * 512:2 * C + fs * 512 + n],
                                     start=(c == 0), stop=(c == NP - 1))
                nc.vector.scalar_tensor_tensor(
                    v3[:, fs * 8:fs * 8 + nh, 0:64],
                    ps[:, 0:n].rearrange("p (h d) -> p h d", d=64),
                    0.0,
                    bv_bc[:, fs * 512:fs * 512 + n].rearrange("p (h d) -> p h d", d=64),
                    AluOpType.add, AluOpType.add)

    def blend_m(gq):
        # am/cm cols [gq*512,(gq+1)*512) select even/odd 128-blocks out of
        # a_bc/c_bc cols [gq*1024,(gq+1)*1024) by the per-core sel vector.
        for src, dst in ((a_bc, am_bc), (c_bc, cm_bc)):
            sview = src[:, gq * 1024:(gq + 1) * 1024].rearrange(
                "p (b e k) -> p b e k", e=2, k=128)
            dview = dst[:, gq * 512:(gq + 1) * 512].rearrange(
                "p (b o k) -> p b o k", o=1, k=128)
            nc.vector.tensor_scalar_mul(dview[:], sview[:, :, 0:1, :],
                                        sel_sb[:, 0:1])
            nc.vector.scalar_tensor_tensor(dview[:], sview[:, :, 1:2, :],
                                           sel_sb[:, 1:2], dview[:],
                                           AluOpType.mult, AluOpType.add)

    def hm_build(gq):
        gsl = slice(gq * 512, (gq + 1) * 512)
        for c in range(NP):
            nc.vector.tensor_mul(xhm[c][:, gsl], xhm[c][:, gsl], am_bc[:, gsl])
            nc.vector.tensor_add(xhm[c][:, gsl], xhm[c][:, gsl], cm_bc[:, gsl])

    def emit_Q(gq):
        gsl = slice(gq * 512, (gq + 1) * 512)
        for f in range(NP):
            ps = gps.tile([128, 512], FP, tag="ps", name="psq")
            for c in range(NP):
                nc.tensor.matmul(ps[:], w_sb[c][:, f * 128:(f + 1) * 128],
                                 xhm[c][:, gsl], start=(c == 0), stop=(c == NP - 1))
            nc.scalar.activation(QT[f][:, gsl], ps[:], AF.Identity,
                                 bias=bias_sb[f][:, 0:1])

    stats_panel(0)
    for f in range(NP):
        nc.sync.dma_start(bias_sb[f][:], d_bias[f * 128:(f + 1) * 128, :])
    nc.sync.dma_start(bv_row[:], d_bvrow[:])
    nc.vector.tensor_copy(bv_rb[:], bv_row[:])
    nc.gpsimd.partition_broadcast(bv_bc[:], bv_rb[:])
    nc.sync.dma_start(sel_sb[:], d_sel[:])
    finalize(0)
    h_build(0)
    for g in range(1, 4):
        stats_panel(g)
        if g == 1:
            # mine-x load + cast; needed first by hm_build at g==2, so keep
            # these scalar casts behind panel 1's on the queue
            for c in range(NP):
                xtm = g_roll.tile([128, TM], FP, tag="xtm", name="xtm")
                nc.sync.dma_start(xtm[:], d_xTm[c * 128:(c + 1) * 128, :])
                nc.scalar.copy(xhm[c][:], xtm[:])
        emit_K(g - 1)
        emit_V(g - 1)
        if g == 2:
            blend_m(0)
            hm_build(0)
            emit_Q(0)
        finalize(g)
        h_build(g)
    emit_K(3)
    emit_V(3)
    blend_m(1)
    hm_build(1)
    emit_Q(1)
    nc.sync.dma_start(mask_a[:], d_masks[0:128, :])
    nc.sync.dma_start(mask_b[:], d_masks[128:256, :])
    nc.sync.dma_start(ident[:], d_masks[256:384, :])

    gps_es.close()
    sps_es.close()
    ln_es.close()

    # ================= attention =================
    # prefetch MLP fc1 weights now (right-side pool; alloc waits on
    # released LN pools)
    w1_es = ExitStack()
    g_w1 = w1_es.enter_context(tc.tile_pool(name="w1p", bufs=1, side="right"))
    w1_sb = [g_w1.tile([128, 4 * C], BF, tag=f"w1_{c}", name=f"w1_{c}") for c in range(NP)]
    for c in range(NP):
        nc.sync.dma_start(w1_sb[c][:], d_w1[c * 128:(c + 1) * 128, :])

    att_es = ExitStack()
    g_wei = att_es.enter_context(tc.tile_pool(name="wei", bufs=9, named_scope="attn"))
    g_rs = att_es.enter_context(tc.tile_pool(name="rspool", bufs=1))
    ps_s_pool = att_es.enter_context(tc.tile_pool(name="sps", bufs=3, space="PSUM"))
    pa_es = ExitStack()
    ps_a_pool = pa_es.enter_context(tc.tile_pool(name="aps", bufs=2, space="PSUM"))

    # row-sums scattered at matmul-legal partitions {0,32,64} x 8 col groups
    rs = g_rs.tile([65, 4096], FP, tag="rs", name="rs")

    def _mask_of(sb, g):
        # diagonal-boundary block of score block sb within query group g,
        # or None. Masks are multiplicative 0/1 tiles applied to wei.
        if (sb - 8 * g) % 2 == 0:
            ja = (sb - 8 * g) // 2
            if 0 <= ja < 4:
                return mask_a, ja
        else:
            jb = (sb - 1 - 8 * g) // 2
            if 0 <= jb < 4:
                return mask_b, jb
        return None

    for hh in range(H):
        ht, hp = hh // 2, (hh % 2) * 64
        pa = {}
        for g in (0, 1):
            pa[g] = ps_a_pool.tile([65, 512], FP, tag="pa", name=f"pa{hh}_{g}")
        pending = []

        def flush_one():
            sb, g, c0, wei, ws = pending.pop(0)
            smax = 8 + 8 * g
            nc.tensor.matmul(pa[g][:, c0:512], Vsb[sb][:, hh * 65:(hh + 1) * 65],
                             wei[:, ws + c0:ws + 512], start=(sb == 0),
                             stop=(sb == smax - 1), skip_group_check=True)

        for sb in range(NT):
            # scores for this key block: query group 0 (cols 0:512 of the
            # pair tile, only sb<8) and group 1 (cols 512:1024), one exp.
            pp = ps_s_pool.tile([128, 1024], FP, tag="pp", name="qk")
            wei = g_wei.tile([128, 1024], BF, tag="wei", name="wei")
            if sb < 8:
                c0 = 128 * max(0, math.ceil((sb - 1) / 2))
                m = _mask_of(sb, 0)
                nc.tensor.matmul(pp[:, c0:512],
                                 KT[ht][hp:hp + 64, sb * 128:(sb + 1) * 128],
                                 QT[ht][hp:hp + 64, c0:512],
                                 start=True, stop=True)
                nc.tensor.matmul(pp[:, 512:1024],
                                 KT[ht][hp:hp + 64, sb * 128:(sb + 1) * 128],
                                 QT[ht][hp:hp + 64, 512:1024],
                                 start=True, stop=True, skip_group_check=True)
                nc.scalar.activation(wei[:, c0:1024], pp[:, c0:1024], AF.Exp,
                                     bias=shift_c[:])
                if m is not None:
                    mt, j = m
                    nc.gpsimd.tensor_mul(wei[:, j * 128:(j + 1) * 128],
                                         wei[:, j * 128:(j + 1) * 128], mt[:])
                pending.append((sb, 0, c0, wei, 0))
                pending.append((sb, 1, 0, wei, 512))
            else:
                c0 = 128 * max(0, math.ceil((sb - 9) / 2))
                m = _mask_of(sb, 1)
                nc.tensor.matmul(pp[:, c0:512],
                                 KT[ht][hp:hp + 64, sb * 128:(sb + 1) * 128],
                                 QT[ht][hp:hp + 64, 512 + c0:1024],
                                 start=True, stop=True)
                nc.scalar.activation(wei[:, c0:512], pp[:, c0:512], AF.Exp,
                                     bias=shift_c[:])
                if m is not None:
                    mt, j = m
                    nc.gpsimd.tensor_mul(wei[:, j * 128:(j + 1) * 128],
                                         wei[:, j * 128:(j + 1) * 128], mt[:])
                pending.append((sb, 1, c0, wei, 0))
            while len(pending) > 3:
                flush_one()
        while pending:
            flush_one()
        for g in (0, 1):
            rr = hh * 2 + g
            row, cg = 32 * (rr % 3), rr // 3
            nc.vector.tensor_copy(attnT[ht][hp:hp + 64, g * 512:(g + 1) * 512],
                                  pa[g][0:64, :])
            nc.vector.tensor_copy(rs[row:row + 1, cg * 512:(cg + 1) * 512],
                                  pa[g][64:65, :])

    pa_es.close()

    # normalization tail: one batched reciprocal, tensor-engine broadcasts
    rb_es = ExitStack()
    rb_pool = rb_es.enter_context(tc.tile_pool(name="rbps", bufs=2, space="PSUM"))
    nc.vector.reciprocal_approx_fast(rs[:], rs[:])
    rsb = g_rs.tile([65, 4096], BF, tag="rsb", name="rsb")
    nc.vector.tensor_copy(rsb[:], rs[:])
    # one [128,512] broadcast tile covers both heads of a KT partition tile
    for ht in range(NP):
        for g in (0, 1):
            rb = rb_pool.tile([128, 512], FP, tag="rb", name="rb")
            for sub in (0, 1):
                rr = (2 * ht + sub) * 2 + g
                row, cg = 32 * (rr % 3), rr // 3
                nc.tensor.matmul(rb[sub * 64:(sub + 1) * 64, :],
                                 ones64_bc[row:row + 1, :],
                                 rsb[row:row + 1, cg * 512:(cg + 1) * 512],
                                 start=True, stop=True, skip_group_check=True)
            nc.vector.tensor_mul(attnT[ht][:, g * 512:(g + 1) * 512],
                                 attnT[ht][:, g * 512:(g + 1) * 512], rb[:])
    rb_es.close()
    att_es.close()
    kqv_es.close()

    # ================= proj + residual + LN2 stats =================
    xmid_es = ExitStack()
    g_xmid = xmid_es.enter_context(tc.tile_pool(name="xmid", bufs=1, side="right"))
    xmid = [g_xmid.tile([128, TM], FP, tag=f"xm{i}", name=f"xm{i}") for i in range(NP)]
    # prefetch MLP fc2 weights now (right-side pool)
    w2_es = ExitStack()
    g_w2 = w2_es.enter_context(tc.tile_pool(name="w2p", bufs=1, side="right"))
    g_xb2 = ExitStack()
    g_h2 = g_xb2.enter_context(tc.tile_pool(name="h2p", bufs=1, side="right"))
    sps2_es = ExitStack()
    sps2 = sps2_es.enter_context(tc.tile_pool(name="statps2", bufs=1, space="PSUM"))
    proj_es = ExitStack()
    gps2 = proj_es.enter_context(tc.tile_pool(name="pps", bufs=3, space="PSUM"))
    g_pr = proj_es.enter_context(tc.tile_pool(name="projroll", bufs=1, named_scope="proj"))
    w2_sb = [g_w2.tile([128, 4 * C], BF, tag=f"w2_{c}", name=f"w2_{c}") for c in range(NP)]
    for c in range(NP):
        nc.sync.dma_start(w2_sb[c][:], d_w2[c * 128:(c + 1) * 128, :])

    stats2 = [sps2.tile([33, 512], FP, tag=f"st2{gq}", name=f"st2{gq}")
              for gq in range(2)]
    xb2 = [g_h2.tile([128, TM], BF, tag=f"xb2_{c}", name=f"xb2_{c}") for c in range(NP)]

    # prefetch residual x for all proj tiles
    xrs = [g_pr.tile([128, TM], FP, tag=f"xr{f}", name=f"xr{f}") for f in range(NP)]
    for f in range(NP):
        nc.sync.dma_start(xrs[f][:], d_xTm[f * 128:(f + 1) * 128, :])
    sq2s = []
    for f in range(NP):
        for g in range(TM // 512):
            ps = gps2.tile([128, 512], FP, tag="ps", name="psp")
            for c in range(NP):
                nc.tensor.matmul(ps[:], w_sb[c][:, 3 * C + f * 128:3 * C + (f + 1) * 128],
                                 attnT[c][:, g * 512:(g + 1) * 512],
                                 start=(c == 0), stop=(c == NP - 1))
            nc.vector.scalar_tensor_tensor(
                xmid[f][:, g * 512:(g + 1) * 512], ps[:], bias_sb[f][:, 3:4],
                xrs[f][:, g * 512:(g + 1) * 512], AluOpType.add, AluOpType.add)
        # casts for LN2 stats run on scalar, off the tensor queue
        nc.scalar.copy(xb2[f][:], xmid[f][:])
        sq2 = g_pr.tile([128, TM], BF, tag=f"sq2_{f}", name=f"sq2_{f}")
        nc.scalar.square(sq2[:], xmid[f][:])
        sq2s.append(sq2)

        def emit_stats2(ff):
            for gq in range(2):
                gsl = slice(gq * 512, (gq + 1) * 512)
                nc.tensor.matmul(stats2[gq][0:1, :], ones_bf[:], xb2[ff][:, gsl],
                                 start=(ff == 0), stop=(ff == NP - 1),
                                 skip_group_check=True)
                nc.tensor.matmul(stats2[gq][32:33, :], ones_bf[:],
                                 sq2s[ff][:, gsl],
                                 start=(ff == 0), stop=(ff == NP - 1),
                                 skip_group_check=True)

        if f > 0:
            emit_stats2(f - 1)
    emit_stats2(NP - 1)
    proj_es.close()
    attnT_es.close()
    w_es.close()

    # ================= LN2 finalize + MLP =================
    mlp_es = ExitStack()
    g_r = mlp_es.enter_context(tc.tile_pool(name="rp", bufs=2, named_scope="mlp"))
    g_roll2 = mlp_es.enter_context(tc.tile_pool(name="mlproll", bufs=2))
    g_bc2 = mlp_es.enter_context(tc.tile_pool(name="mlpbc", bufs=1))
    g_small2 = mlp_es.enter_context(tc.tile_pool(name="mlpsmall", bufs=1))
    gps3 = mlp_es.enter_context(tc.tile_pool(name="mps", bufs=3, space="PSUM"))

    a_bc2 = g_bc2.tile([128, TM], BF, tag="a2bc", name="a2bc")
    c_bc2 = g_bc2.tile([128, TM], BF, tag="c2bc", name="c2bc")

    def finalize2(gq):
        gsl = slice(gq * 512, (gq + 1) * 512)
        mu = g_small2.tile([1, 512], FP, tag="mu2", name="mu2")
        nc.scalar.mul(mu[:], stats2[gq][0:1, :], 1.0 / C)
        m2 = g_small2.tile([1, 512], FP, tag="m22", name="m22")
        nc.scalar.mul(m2[:], stats2[gq][32:33, :], 1.0 / C)
        va = g_small2.tile([1, 512], FP, tag="va2", name="va2")
        nc.vector.tensor_mul(va[:], mu[:], mu[:])
        nc.vector.scalar_tensor_tensor(va[:], m2[:], EPS, va[:],
                                       AluOpType.add, AluOpType.subtract)
        nc.vector.reciprocal_approx_fast(va[:], va[:])
        rstd = g_small2.tile([1, 512], FP, tag="rstd2", name="rstd2")
        nc.scalar.activation(rstd[:], va[:], AF.Sqrt)
        nc.vector.scalar_tensor_tensor(mu[:], mu[:], -1.0, rstd[:],
                                       AluOpType.mult, AluOpType.mult)
        a5b = g_small2.tile([1, 512], BF, tag="a5b2", name="a5b2")
        nc.vector.tensor_copy(a5b[:], rstd[:])
        c5b = g_small2.tile([1, 512], BF, tag="c5b2", name="c5b2")
        nc.vector.tensor_copy(c5b[:], mu[:])
        nc.gpsimd.partition_broadcast(a_bc2[:, gsl], a5b[:])
        nc.gpsimd.partition_broadcast(c_bc2[:, gsl], c5b[:])

    def h2_build(gq):
        gsl = slice(gq * 512, (gq + 1) * 512)
        for c in range(NP):
            nc.vector.tensor_mul(xb2[c][:, gsl], xb2[c][:, gsl], a_bc2[:, gsl])
            nc.vector.tensor_add(xb2[c][:, gsl], xb2[c][:, gsl], c_bc2[:, gsl])

    finalize2(0)
    h2_build(0)
    finalize2(1)
    h2_build(1)

    for g in range(TM // 512):
        r_tiles = []
        for m in range(24):
            ps = gps3.tile([128, 512], FP, tag="ps", name="ps1")
            for c in range(NP):
                nc.tensor.matmul(ps[:], w1_sb[c][:, m * 128:(m + 1) * 128],
                                 xb2[c][:, g * 512:(g + 1) * 512],
                                 start=(c == 0), stop=(c == NP - 1))
            r = g_r.tile([128, 512], BF, tag=f"r{m}", name=f"r{m}")
            nc.scalar.activation(r[:], ps[:], AF.Relu,
                                 bias=bias_sb[m % 6][:, 5 + m // 6:6 + m // 6])
            r_tiles.append(r)
        for f in range(NP):
            ps = gps3.tile([128, 512], FP, tag="ps", name="ps2")
            for m in range(24):
                nc.tensor.matmul(ps[:], w2_sb[m // 4][:, (m % 4) * 768 + f * 128:
                                                      (m % 4) * 768 + (f + 1) * 128],
                                 r_tiles[m][:],
                                 start=(m == 0), stop=(m == 23))
            ot = g_roll2.tile([128, 512], FP, tag="ot", name="ot")
            nc.vector.scalar_tensor_tensor(ot[:], ps[:], bias_sb[f][:, 4:5],
                                           xmid[f][:, g * 512:(g + 1) * 512],
                                           AluOpType.add, AluOpType.add)
            nc.sync.dma_start(d_out[f * 128:(f + 1) * 128, g * 512:(g + 1) * 512],
                              ot[:])
    mlp_es.close()
    sps2_es.close()
    g_xb2.close()
    w2_es.close()
    xmid_es.close()
    w1_es.close()
    es.close()


# ---------------------------------------------------------------------------
# host side
# ---------------------------------------------------------------------------

def _mycols(half):
    blocks = np.arange(8) * 2 + half
    return (blocks[:, None] * 128 + np.arange(128)[None, :]).reshape(-1)


def _prep_inputs(x, wq, bq, wk, bk, wv, bv, w_proj, b_proj, w1, b1, w2, b2,
                 g1, beta1, g2, beta2):
    x = np.asarray(x, np.float32)
    wq_f = np.ascontiguousarray(np.transpose(np.asarray(wq, np.float32), (1, 0, 2)).reshape(C, C))
    wk_f = np.ascontiguousarray(np.transpose(np.asarray(wk, np.float32), (1, 0, 2)).reshape(C, C))
    wv_f = np.ascontiguousarray(np.transpose(np.asarray(wv, np.float32), (1, 0, 2)).reshape(C, C))
    g1 = np.asarray(g1, np.float32); beta1 = np.asarray(beta1, np.float32)
    g2 = np.asarray(g2, np.float32); beta2 = np.asarray(beta2, np.float32)
    w1 = np.asarray(w1, np.float32); w2 = np.asarray(w2, np.float32)
    w_proj = np.asarray(w_proj, np.float32)

    wq_g = g1[:, None] * wq_f
    wk_g = g1[:, None] * wk_f
    wv_g = g1[:, None] * wv_f
    bq_f = beta1 @ wq_f + np.asarray(bq, np.float32).reshape(-1)
    bk_f = beta1 @ wk_f + np.asarray(bk, np.float32).reshape(-1)
    bv_f = beta1 @ wv_f + np.asarray(bv, np.float32).reshape(-1)
    w1_g = g2[:, None] * w1
    b1_f = beta2 @ w1 + np.asarray(b1, np.float32)

    wqkvp = np.concatenate([wq_g, wk_g, wv_g, w_proj], axis=1).astype(bf16)
    w1p = w1_g.astype(bf16)
    w2p = np.ascontiguousarray(
        w2.reshape(6, 4, 128, C).transpose(0, 2, 1, 3).reshape(C, 4 * C)).astype(bf16)

    biasp = np.zeros((C, 9), np.float32)
    biasp[:, 0] = bq_f
    biasp[:, 1] = bk_f
    biasp[:, 2] = bv_f
    biasp[:, 3] = np.asarray(b_proj, np.float32)
    biasp[:, 4] = np.asarray(b2, np.float32)
    biasp[:, 5:9] = b1_f.reshape(4, C).T
    bvrow = bv_f.reshape(1, C).astype(np.float32)

    tri = np.tril(np.ones((128, 128), np.float32)).T  # [s, q]: 1 iff s <= q
    NEG = -30000.0
    in_maps = []
    for core in range(8):
        b, half = core // 2, core % 2
        xT = np.ascontiguousarray(x[b].T)
        xTm = np.ascontiguousarray(xT[:, _mycols(half)])
        # multiplicative mask tiles: 1 where allowed, 0 where masked
        masks = np.zeros((384, 128), np.float32)
        if half == 0:
            masks[0:128] = tri
            masks[128:256] = 0.0
        else:
            masks[0:128] = 1.0
            masks[128:256] = tri
        masks[256:384] = np.eye(128, dtype=np.float32)
        sel = np.zeros((128, 2), np.float32)
        sel[:, 0] = 1.0 - half
        sel[:, 1] = half
        in_maps.append({
            "xT": xT, "xTm": xTm,
            "wqkvp": wqkvp, "w1p": w1p, "w2p": w2p,
            "biasp": biasp, "bvrow": bvrow,
            "masks": masks.astype(bf16),
            "sel": sel,
        })
    return in_maps


def _assemble(results, dtype):
    out = np.empty((B, T, C), dtype)
    for core in range(8):
        b, half = core // 2, core % 2
        out[b, _mycols(half), :] = results[core]["outT"].T
    return out


def kernel(**inputs):
    in_maps = _prep_inputs(**inputs)
    if "nc" not in _cache:
        _cache["nc"] = _build()
    res = bass_utils.run_bass_kernel_spmd(_cache["nc"], in_maps,
                                          core_ids=list(range(8)))
    return _assemble(res.results, np.asarray(inputs["x"]).dtype)
